# revision 4
# baseline (speedup 1.0000x reference)
"""Trainium2 Bass kernel for nn_CGRModel (6-branch MHA + PLE gates + 3 heads).

Self-contained: hardcodes all shapes. Data-parallel over batch B=8 across the
8 NeuronCores (1 sequence per core). All activations are kept transposed
[feature, L] on chip; weights are host-pre-transposed to [in, out] so weight
tiles serve directly as the matmul stationary operand. Attention scores are
computed transposed (S^T: k on partitions, q on free) so softmax numerators
feed attn@V with no on-chip transposes; softmax skips max-subtraction (score
scale ~0.2); denominators are M=1 ones-column matmuls accumulated in PSUM.
"""
import numpy as np

P = 128
D = 512
L = 1024
NKT = 4    # feature tiles of 128 (D/P)
NLT = 2    # 512-wide L tiles
NMT = 8    # 128-wide L tiles
NPAIR = 4  # head pairs (8 heads, dh=64)
W4, W6 = 2, 3

_CACHE = {}


def _blocks_for(kind, qt, W=None):
    if kind == "causal":
        out = []
        for kb in range(4 * qt + 4):
            qoff = max(0, 128 * (kb - 4 * qt))
            out.append((kb, qoff, 512 - qoff, "tri" if kb >= 4 * qt else None))
        return out
    if kind == "full":
        return [(kb, 0, 512, None) for kb in range(8)]
    if kind == "band":
        out = []
        for i, kb in enumerate(range(4 * qt, 4 * qt + 4)):
            qoff = 128 * i
            out.append((kb, qoff, min(128 + W, 512 - qoff), "band"))
        return out
    raise ValueError(kind)


def _build_program():
    import concourse.mybir as mybir
    from concourse import bacc
    from concourse.tile import TileContext

    F32 = mybir.dt.float32
    AF = mybir.ActivationFunctionType

    nc = bacc.Bacc()
    dp = nc.declare_dram_parameter
    xt_p = dp("xt", [D, L], F32, isOutput=False)
    ut_p = dp("ut", [D, L], F32, isOutput=False)
    wqkv_p = dp("wqkv", [6, D, 3 * D], F32, isOutput=False)
    wo_p = dp("wo", [6, D, D], F32, isOutput=False)
    bqkv_p = dp("bqkv", [P, 72], F32, isOutput=False)
    bo_p = dp("bo", [6, 1, D], F32, isOutput=False)
    bv_p = dp("bv", [6, 1, D], F32, isOutput=False)
    wcross_p = dp("wcross", [D, 2 * D], F32, isOutput=False)
    bcross_p = dp("bcross", [P, 8], F32, isOutput=False)
    wgate_p = dp("wgate", [2, D, 6], F32, isOutput=False)
    bg_p = dp("bg", [6, 2], F32, isOutput=False)
    wh1_p = dp("wh1", [3, D, 256], F32, isOutput=False)
    bh1_p = dp("bh1", [P, 6], F32, isOutput=False)
    wh2_p = dp("wh2", [P, 6], F32, isOutput=False)
    bh2_p = dp("bh2", [1, 3], F32, isOutput=False)
    ones_p = dp("ones", [P, 512], F32, isOutput=False)
    tri_p = dp("tri", [P, P], F32, isOutput=False)
    band4_p = dp("band4", [P, 128 + W4], F32, isOutput=False)
    band6_p = dp("band6", [P, 128 + W6], F32, isOutput=False)
    cb4_p = dp("cb4", [P, W4], F32, isOutput=False)
    cb6_p = dp("cb6", [P, W6], F32, isOutput=False)
    ind6_p = dp("ind6", [6, 6 * P], F32, isOutput=False)
    out_p = dp("out", [3, L], F32, isOutput=True)

    with TileContext(nc) as tc:
        with (
            tc.tile_pool(name="sb", bufs=1) as sb,
            tc.tile_pool(name="ps", bufs=1, space="PSUM") as psp,
        ):
            # ---------------- constants + persistent activations ----------
            ones = sb.tile([P, 512], F32, tag="ones")
            tri = sb.tile([P, P], F32, tag="tri")
            band4 = sb.tile([P, 128 + W4], F32, tag="band4")
            band6 = sb.tile([P, 128 + W6], F32, tag="band6")
            cb4 = sb.tile([P, W4], F32, tag="cb4")
            cb6 = sb.tile([P, W6], F32, tag="cb6")
            ind6 = sb.tile([6, 6 * P], F32, tag="ind6")
            bqkv = sb.tile([P, 72], F32, tag="bqkv")
            bcross = sb.tile([P, 8], F32, tag="bcross")
            bg6 = sb.tile([6, 2], F32, tag="bg6")
            bh1s = sb.tile([P, 6], F32, tag="bh1s")
            wh2s = sb.tile([P, 6], F32, tag="wh2s")
            bh2s = sb.tile([1, 3], F32, tag="bh2s")
            for t_, p_ in ((ones, ones_p), (tri, tri_p), (band4, band4_p),
                           (band6, band6_p), (cb4, cb4_p), (cb6, cb6_p),
                           (ind6, ind6_p), (bqkv, bqkv_p), (bcross, bcross_p),
                           (bg6, bg_p), (bh1s, bh1_p), (wh2s, wh2_p),
                           (bh2s, bh2_p)):
                nc.sync.dma_start(out=t_, in_=p_[:, :])

            xt = [sb.tile([P, L], F32, tag="xt", bufs=4, name=f"xt{k}")
                  for k in range(NKT)]
            for k in range(NKT):
                nc.sync.dma_start(out=xt[k], in_=xt_p[128 * k:128 * k + 128, :])
            hexp = [sb.tile([P, L], F32, tag="hexp", bufs=4, name=f"hexp{k}")
                    for k in range(NKT)]
            hclk = [sb.tile([P, L], F32, tag="hclk", bufs=4, name=f"hclk{k}")
                    for k in range(NKT)]
            htasks = [hexp, hclk]

            # ---------------- gates ---------------------------------------
            g6 = []
            for t in range(2):
                egs = []
                wg = [sb.tile([P, 6], F32, tag="wg", bufs=8, name=f"wg{t}{k}")
                      for k in range(NKT)]
                for k in range(NKT):
                    nc.sync.dma_start(out=wg[k],
                                      in_=wgate_p[t, 128 * k:128 * k + 128, :])
                for lt in range(NLT):
                    gps = psp.tile([P, 512], F32, tag="mm", bufs=2,
                                   name=f"gps{t}{lt}")
                    for k in range(NKT):
                        nc.tensor.matmul(gps[0:6, 0:512], wg[k][:, 0:6],
                                         xt[k][:, 512 * lt:512 * lt + 512],
                                         start=(k == 0), stop=(k == NKT - 1))
                    eg = sb.tile([6, 512], F32, tag="eg", bufs=2,
                                 name=f"eg{t}{lt}")
                    egs.append(eg)
                    nc.scalar.activation(eg[0:6, :], gps[0:6, 0:512], AF.Exp,
                                         bias=bg6[0:6, t:t + 1])
                dg = psp.tile([1, L], F32, tag="den", bufs=1, name=f"dg{t}")
                for lt in range(NLT):
                    nc.tensor.matmul(dg[0:1, 512 * lt:512 * lt + 512],
                                     ones[0:6, 0:1], egs[lt][0:6, :],
                                     start=True, stop=True,
                                     tile_position=(0, 0))
                rg = sb.tile([1, L], F32, tag="rg", bufs=1, name=f"rg{t}")
                nc.vector.reciprocal(rg[0:1, :], dg[0:1, :])
                g6t = sb.tile([6, L], F32, tag="g6", bufs=2, name=f"g6{t}")
                for lt in range(NLT):
                    rgb = sb.tile([6, 512], F32, tag="rgb", bufs=1,
                                  name=f"rgb{t}{lt}")
                    nc.gpsimd.partition_broadcast(
                        rgb[0:6, :], rg[0:1, 512 * lt:512 * lt + 512])
                    nc.vector.tensor_mul(g6t[0:6, 512 * lt:512 * lt + 512],
                                         egs[lt][0:6, :], rgb[0:6, :])
                g6.append(g6t)

            # ---------------- helpers -------------------------------------
            def emit_gating(expert, e):
                for t in range(2):
                    for lt in range(NLT):
                        ls = slice(512 * lt, 512 * lt + 512)
                        bcp = psp.tile([P, 512], F32, tag="mm", bufs=2,
                                       name=f"gb{e}{t}{lt}")
                        nc.tensor.matmul(bcp[:, 0:512],
                                         ind6[0:6, 128 * e:128 * e + 128],
                                         g6[t][0:6, ls], start=True, stop=True,
                                         tile_position=(0, 0))
                        for k in range(NKT):
                            if e == 0:
                                nc.vector.tensor_mul(htasks[t][k][:, ls],
                                                     expert[k][:, ls], bcp)
                            else:
                                tmp = sb.tile([P, 512], F32, tag="e", bufs=3,
                                              name=f"gt{e}{t}{lt}{k}")
                                nc.vector.tensor_mul(tmp, expert[k][:, ls], bcp)
                                nc.vector.tensor_add(htasks[t][k][:, ls],
                                                     htasks[t][k][:, ls], tmp)

            def emit_attn(a, kind, qsrc, kvsrc, resid, out_tiles, e, W=None):
                # weights
                w_sb = [sb.tile([P, 3 * D], F32, tag="wqkv", bufs=4,
                                name=f"wqkv{a}_{k}") for k in range(NKT)]
                wo_sb = [sb.tile([P, D], F32, tag="wo", bufs=4,
                                 name=f"wo{a}_{k}") for k in range(NKT)]
                for k in range(NKT):
                    nc.sync.dma_start(
                        out=w_sb[k], in_=wqkv_p[a, 128 * k:128 * k + 128, :])
                    nc.sync.dma_start(
                        out=wo_sb[k], in_=wo_p[a, 128 * k:128 * k + 128, :])
                bor = sb.tile([1, D], F32, tag="bor", bufs=1, name=f"bor{a}")
                nc.sync.dma_start(out=bor, in_=bo_p[a])
                bvr = sb.tile([1, D], F32, tag="bvr", bufs=1, name=f"bvr{a}")
                nc.sync.dma_start(out=bvr, in_=bv_p[a])
                bvbc = sb.tile([P, 512], F32, tag="bc", bufs=2,
                               name=f"bvbc{a}")
                nc.gpsimd.partition_broadcast(bvbc[:, :], bvr[0:1, :])

                # V in natural layout [l, feat]
                vnat = []
                for mt in range(NMT):
                    vps = psp.tile([P, 512], F32, tag="mm", bufs=2,
                                   name=f"vps{a}{mt}")
                    for k in range(NKT):
                        nc.tensor.matmul(
                            vps, kvsrc[k][:, 128 * mt:128 * mt + 128],
                            w_sb[k][:, 2 * D:3 * D],
                            start=(k == 0), stop=(k == NKT - 1))
                    vt = sb.tile([P, 512], F32, tag="v", bufs=8,
                                 name=f"v{a}{mt}")
                    nc.vector.tensor_add(vt, vps, bvbc)
                    vnat.append(vt)

                mask_t = {"tri": tri, "band": band4 if W == W4 else band6}
                cmask = cb4 if W == W4 else cb6
                cat = []
                for t in range(NPAIR):
                    # Q^T / K^T for this head pair (feature rows 128t..)
                    qta = sb.tile([P, L], F32, tag="qk", bufs=4,
                                  name=f"q{a}{t}")
                    kta = sb.tile([P, L], F32, tag="qk", bufs=4,
                                  name=f"k{a}{t}")
                    for dst, coff, bcol, src in (
                            (qta, 128 * t, 12 * a + t, qsrc),
                            (kta, D + 128 * t, 12 * a + 4 + t, kvsrc)):
                        for lt in range(NLT):
                            ls = slice(512 * lt, 512 * lt + 512)
                            ips = psp.tile([P, 512], F32, tag="mm", bufs=2,
                                           name=f"ip{a}{t}{lt}")
                            for k in range(NKT):
                                nc.tensor.matmul(
                                    ips, w_sb[k][:, coff:coff + 128],
                                    src[k][:, ls],
                                    start=(k == 0), stop=(k == NKT - 1))
                            nc.vector.tensor_scalar_add(
                                dst[:, ls], ips, bqkv[:, bcol:bcol + 1])
                    ct = sb.tile([P, L], F32, tag="cat", bufs=4,
                                 name=f"cat{a}{t}")
                    cat.append(ct)
                    for qt in range(2):
                        blocks = _blocks_for(kind, qt, W)
                        corner = (kind == "band" and qt == 1)
                        ngrp = len(blocks) + (1 if corner else 0)
                        U = psp.tile([P, 512], F32, tag="u", bufs=2,
                                     name=f"U{a}{t}{qt}")
                        den = psp.tile([1, L], F32, tag="den", bufs=1,
                                       name=f"den{a}{t}{qt}")
                        for bi, (kb, qoff, w, mk) in enumerate(blocks):
                            st = bi == 0
                            sp = bi == ngrp - 1
                            qs = slice(512 * qt + qoff, 512 * qt + qoff + w)
                            sA = psp.tile([P, 512], F32, tag="s", bufs=2,
                                          name=f"sA{a}{t}{qt}{kb}")
                            sB = psp.tile([P, 512], F32, tag="s", bufs=2,
                                          name=f"sB{a}{t}{qt}{kb}")
                            ks = slice(128 * kb, 128 * kb + 128)
                            nc.tensor.matmul(sA[:, 0:w], kta[0:64, ks],
                                             qta[0:64, qs],
                                             start=True, stop=True)
                            nc.tensor.matmul(sB[:, 0:w], kta[64:128, ks],
                                             qta[64:128, qs],
                                             start=True, stop=True,
                                             tile_position=(64, 0))
                            eA = sb.tile([P, 512], F32, tag="e", bufs=3,
                                         name=f"eA{a}{t}{qt}{kb}")
                            eB = sb.tile([P, 512], F32, tag="e", bufs=3,
                                         name=f"eB{a}{t}{qt}{kb}")
                            nc.scalar.activation(eA[:, 0:w], sA[:, 0:w], AF.Exp)
                            nc.scalar.activation(eB[:, 0:w], sB[:, 0:w], AF.Exp)
                            if mk == "tri":
                                nc.vector.tensor_mul(eA[:, 0:128],
                                                     eA[:, 0:128], tri)
                                nc.vector.tensor_mul(eB[:, 0:128],
                                                     eB[:, 0:128], tri)
                            elif mk == "band":
                                m = mask_t["band"]
                                nc.vector.tensor_mul(eA[:, 0:w], eA[:, 0:w],
                                                     m[:, 0:w])
                                nc.vector.tensor_mul(eB[:, 0:w], eB[:, 0:w],
                                                     m[:, 0:w])
                            qo = slice(qoff, qoff + w)
                            qo2 = slice(512 + qoff, 512 + qoff + w)
                            nc.tensor.matmul(U[0:64, qo],
                                             vnat[kb][:, 128 * t:128 * t + 64],
                                             eA[:, 0:w], start=st, stop=sp,
                                             tile_position=(0, 0))
                            nc.tensor.matmul(U[64:128, qo],
                                             vnat[kb][:, 128 * t + 64:
                                                      128 * t + 128],
                                             eB[:, 0:w], start=st, stop=sp,
                                             tile_position=(0, 64))
                            nc.tensor.matmul(den[0:1, qo], ones[:, 0:1],
                                             eA[:, 0:w], start=st, stop=sp,
                                             tile_position=(0, 0))
                            nc.tensor.matmul(den[0:1, qo2], ones[:, 0:1],
                                             eB[:, 0:w], start=st, stop=sp,
                                             tile_position=(0, 0))
                        if corner:
                            Wc = W
                            sCA = psp.tile([P, 512], F32, tag="s", bufs=2,
                                           name=f"sCA{a}{t}")
                            sCB = psp.tile([P, 512], F32, tag="s", bufs=2,
                                           name=f"sCB{a}{t}")
                            nc.tensor.matmul(sCA[96:128, 0:Wc],
                                             kta[0:64, 480:512],
                                             qta[0:64, 512:512 + Wc],
                                             start=True, stop=True,
                                             tile_position=(0, 96))
                            nc.tensor.matmul(sCB[96:128, 0:Wc],
                                             kta[64:128, 480:512],
                                             qta[64:128, 512:512 + Wc],
                                             start=True, stop=True,
                                             tile_position=(64, 96))
                            eCA = sb.tile([P, 512], F32, tag="e", bufs=3,
                                          name=f"eCA{a}{t}")
                            eCB = sb.tile([P, 512], F32, tag="e", bufs=3,
                                          name=f"eCB{a}{t}")
                            nc.scalar.activation(eCA[96:128, 0:Wc],
                                                 sCA[96:128, 0:Wc], AF.Exp)
                            nc.scalar.activation(eCB[96:128, 0:Wc],
                                                 sCB[96:128, 0:Wc], AF.Exp)
                            nc.vector.tensor_mul(eCA[96:128, 0:Wc],
                                                 eCA[96:128, 0:Wc],
                                                 cmask[96:128, 0:Wc])
                            nc.vector.tensor_mul(eCB[96:128, 0:Wc],
                                                 eCB[96:128, 0:Wc],
                                                 cmask[96:128, 0:Wc])
                            nc.tensor.matmul(
                                U[0:64, 0:Wc],
                                vnat[3][96:128, 128 * t:128 * t + 64],
                                eCA[96:128, 0:Wc], start=False, stop=True,
                                tile_position=(96, 0))
                            nc.tensor.matmul(
                                U[64:128, 0:Wc],
                                vnat[3][96:128, 128 * t + 64:128 * t + 128],
                                eCB[96:128, 0:Wc], start=False, stop=True,
                                tile_position=(96, 64))
                            nc.tensor.matmul(den[0:1, 0:Wc],
                                             ones[96:128, 0:1],
                                             eCA[96:128, 0:Wc],
                                             start=False, stop=True,
                                             tile_position=(96, 0))
                            nc.tensor.matmul(den[0:1, 512:512 + Wc],
                                             ones[96:128, 0:1],
                                             eCB[96:128, 0:Wc],
                                             start=False, stop=True,
                                             tile_position=(96, 0))
                        rA = sb.tile([1, 512], F32, tag="ra", bufs=1,
                                     name=f"rA{a}{t}{qt}")
                        rB = sb.tile([1, 512], F32, tag="rb", bufs=1,
                                     name=f"rB{a}{t}{qt}")
                        nc.vector.reciprocal(rA[0:1, :], den[0:1, 0:512])
                        nc.vector.reciprocal(rB[0:1, :], den[0:1, 512:1024])
                        bcA = sb.tile([P, 512], F32, tag="bc", bufs=2,
                                      name=f"bcA{a}{t}{qt}")
                        bcB = sb.tile([P, 512], F32, tag="bc", bufs=2,
                                      name=f"bcB{a}{t}{qt}")
                        nc.gpsimd.partition_broadcast(bcA[:, :], rA[0:1, :])
                        nc.gpsimd.partition_broadcast(bcB[:, :], rB[0:1, :])
                        qts = slice(512 * qt, 512 * qt + 512)
                        nc.vector.tensor_mul(ct[0:64, qts], U[0:64, 0:512],
                                             bcA[0:64, :])
                        nc.vector.tensor_mul(ct[64:128, qts], U[64:128, 0:512],
                                             bcB[64:128, :])
                # out_proj (+bias via K=1 ones matmul) + residual
                for ot in range(NKT):
                    for lt in range(NLT):
                        ls = slice(512 * lt, 512 * lt + 512)
                        ops = psp.tile([P, 512], F32, tag="mm", bufs=2,
                                       name=f"op{a}{ot}{lt}")
                        for k in range(NKT):
                            nc.tensor.matmul(
                                ops, wo_sb[k][:, 128 * ot:128 * ot + 128],
                                cat[k][:, ls],
                                start=(k == 0), stop=False)
                        nc.tensor.matmul(ops, bor[0:1, 128 * ot:128 * ot + 128],
                                         ones[0:1, 0:512],
                                         start=False, stop=True,
                                         tile_position=(0, 0))
                        if resid is not None:
                            nc.vector.tensor_add(out_tiles[ot][:, ls], ops,
                                                 resid[ot][:, ls])
                        else:
                            nc.vector.tensor_copy(out_tiles[ot][:, ls], ops)
                emit_gating(out_tiles, e)

            # ---------------- the six attentions --------------------------
            sharedT = [sb.tile([P, L], F32, tag="big", bufs=4,
                               name=f"sh{k}") for k in range(NKT)]
            emit_attn(0, "causal", xt, xt, xt, sharedT, 0)
            e4 = [sb.tile([P, L], F32, tag="exp", bufs=4, name=f"e4_{k}")
                  for k in range(NKT)]
            emit_attn(4, "band", xt, xt, xt, e4, 4, W=W4)
            e1 = [sb.tile([P, L], F32, tag="exp", bufs=4, name=f"e1_{k}")
                  for k in range(NKT)]
            emit_attn(1, "causal", sharedT, sharedT, sharedT, e1, 1)
            e5 = [sb.tile([P, L], F32, tag="exp", bufs=4, name=f"e5_{k}")
                  for k in range(NKT)]
            emit_attn(5, "band", xt, xt, xt, e5, 5, W=W6)
            e2 = [sb.tile([P, L], F32, tag="exp", bufs=4, name=f"e2_{k}")
                  for k in range(NKT)]
            emit_attn(2, "causal", sharedT, sharedT, sharedT, e2, 2)

            # cross projections u = user @ cuW.T + b ; pv = item @ ciW.T + b
            ut = [sb.tile([P, L], F32, tag="exp", bufs=4, name=f"ut{k}")
                  for k in range(NKT)]
            for k in range(NKT):
                nc.sync.dma_start(out=ut[k], in_=ut_p[128 * k:128 * k + 128, :])
            wc_sb = [sb.tile([P, 2 * D], F32, tag="wqkv", bufs=4,
                             name=f"wc{k}") for k in range(NKT)]
            for k in range(NKT):
                nc.sync.dma_start(out=wc_sb[k],
                                  in_=wcross_p[128 * k:128 * k + 128, :])
            def emit_cross_proj(dst, coff, bcol, src):
                for ot in range(NKT):
                    for lt in range(NLT):
                        ls = slice(512 * lt, 512 * lt + 512)
                        cps = psp.tile([P, 512], F32, tag="mm", bufs=2,
                                       name=f"cp{coff}{ot}{lt}")
                        for k in range(NKT):
                            nc.tensor.matmul(
                                cps,
                                wc_sb[k][:, coff + 128 * ot:coff + 128 * ot + 128],
                                src[k][:, ls],
                                start=(k == 0), stop=(k == NKT - 1))
                        nc.vector.tensor_scalar_add(
                            dst[ot][:, ls], cps,
                            bcross[:, bcol + ot:bcol + ot + 1])

            uT = [sb.tile([P, L], F32, tag="big", bufs=4, name=f"uT{k}")
                  for k in range(NKT)]
            emit_cross_proj(uT, 0, 0, ut)
            pvT = [sb.tile([P, L], F32, tag="exp", bufs=4, name=f"pvT{k}")
                   for k in range(NKT)]
            emit_cross_proj(pvT, D, 4, xt)
            e3 = [sb.tile([P, L], F32, tag="exp", bufs=4, name=f"e3_{k}")
                  for k in range(NKT)]
            emit_attn(3, "full", uT, pvT, None, e3, 3)

            # ---------------- heads ---------------------------------------
            for i, hsrc in ((0, hexp), (1, hclk), (2, hclk)):
                wh1 = [sb.tile([P, 256], F32, tag="wo", bufs=4,
                               name=f"wh1_{i}{k}") for k in range(NKT)]
                for k in range(NKT):
                    nc.sync.dma_start(out=wh1[k],
                                      in_=wh1_p[i, 128 * k:128 * k + 128, :])
                hid = [sb.tile([P, L], F32, tag="cat", bufs=4,
                               name=f"hid{i}{m}") for m in range(2)]
                for mt in range(2):
                    for lt in range(NLT):
                        ls = slice(512 * lt, 512 * lt + 512)
                        hps = psp.tile([P, 512], F32, tag="mm", bufs=2,
                                       name=f"hp{i}{mt}{lt}")
                        for k in range(NKT):
                            nc.tensor.matmul(
                                hps, wh1[k][:, 128 * mt:128 * mt + 128],
                                hsrc[k][:, ls],
                                start=(k == 0), stop=(k == NKT - 1))
                        nc.scalar.activation(hid[mt][:, ls], hps, AF.Relu,
                                             bias=bh1s[:, 2 * i + mt:
                                                       2 * i + mt + 1])
                pps = psp.tile([1, L], F32, tag="den", bufs=1, name=f"pp{i}")
                for lt in range(NLT):
                    ls = slice(512 * lt, 512 * lt + 512)
                    for mt in range(2):
                        nc.tensor.matmul(pps[0:1, ls],
                                         wh2s[:, mt * 3 + i:mt * 3 + i + 1],
                                         hid[mt][:, ls],
                                         start=(mt == 0), stop=(mt == 1),
                                         tile_position=(0, 0))
                psb = sb.tile([1, L], F32, tag="p", bufs=1, name=f"psb{i}")
                nc.scalar.activation(psb[0:1, :], pps[0:1, :], AF.Sigmoid,
                                     bias=bh2s[0:1, i:i + 1])
                nc.sync.dma_start(out=out_p[i:i + 1, :], in_=psb)

    nc.finalize()
    return nc


def _host_prep(inputs):
    """Build the shared (weight/const) input map pieces + per-core slices."""
    f32 = np.float32
    w_in = np.asarray(inputs["attn_w_in"], f32)      # [6, 1536, 512]
    b_in = np.asarray(inputs["attn_b_in"], f32)      # [6, 1536]
    w_out = np.asarray(inputs["attn_w_out"], f32)    # [6, 512, 512]
    b_out = np.asarray(inputs["attn_b_out"], f32)    # [6, 512]

    wqkv = np.ascontiguousarray(np.transpose(w_in, (0, 2, 1)))  # [6,512,1536]
    wqkv[:, :, 0:512] /= 8.0                         # fold 1/sqrt(dh) into Wq
    bqkv = b_in.reshape(6, 12, 128)                  # [6, 12f, 128]
    bqkv = bqkv.copy()
    bqkv[:, 0:4, :] /= 8.0
    # device layout [128, 72]: col a*12 + j  (j = feature-tile: 0-3 q, 4-7 k, 8-11 v)
    bqkv_dev = np.ascontiguousarray(bqkv.transpose(2, 0, 1).reshape(128, 72))

    wo = np.ascontiguousarray(np.transpose(w_out, (0, 2, 1)))   # [6,512,512]
    bo = np.ascontiguousarray(b_out[:, None, :])                # [6,1,512]
    bv = np.ascontiguousarray(b_in[:, None, 1024:1536])         # [6,1,512]

    cu = np.asarray(inputs["cross_user_w"], f32)
    ci = np.asarray(inputs["cross_item_w"], f32)
    wcross = np.ascontiguousarray(np.concatenate([cu.T, ci.T], axis=1))
    bcross = np.ascontiguousarray(
        np.concatenate([np.asarray(inputs["cross_user_b"], f32)
                        .reshape(4, 128).T,
                        np.asarray(inputs["cross_item_b"], f32)
                        .reshape(4, 128).T], axis=1))

    wgate = np.ascontiguousarray(np.asarray(inputs["gate_w"], f32))
    bg = np.ascontiguousarray(np.asarray(inputs["gate_b"], f32).T)  # [6,2]
    wh1 = np.ascontiguousarray(np.asarray(inputs["head_w1"], f32))
    bh1 = np.ascontiguousarray(
        np.asarray(inputs["head_b1"], f32).reshape(3, 2, 128)
        .transpose(2, 0, 1).reshape(128, 6))
    # wh2 device [128, 6]: col kt*3 + i
    wh2 = np.ascontiguousarray(
        np.asarray(inputs["head_w2"], f32).reshape(3, 2, 128)
        .transpose(2, 1, 0).reshape(128, 6))
    bh2 = np.ascontiguousarray(np.asarray(inputs["head_b2"], f32)[None, :])

    k_i = np.arange(128)[:, None]
    tri = (np.arange(128)[None, :] >= k_i).astype(f32)
    def band_mask(W):
        c = np.arange(128 + W)[None, :]
        return ((c - k_i >= 0) & (c - k_i <= W)).astype(f32)
    def corner_mask(W):
        m = np.zeros((128, W), f32)
        r = np.arange(96, 128)[:, None] - 96
        c = np.arange(W)[None, :]
        m[96:128, :] = (32 + c - r <= W).astype(f32)
        return m
    ind6 = np.zeros((6, 768), f32)
    for e in range(6):
        ind6[e, 128 * e:128 * e + 128] = 1.0

    shared = {
        "wqkv": wqkv, "bqkv": bqkv_dev, "wo": wo, "bo": bo, "bv": bv,
        "wcross": wcross, "bcross": bcross, "wgate": wgate, "bg": bg,
        "wh1": wh1, "bh1": bh1, "wh2": wh2, "bh2": bh2,
        "ones": np.ones((128, 512), f32), "tri": tri,
        "band4": band_mask(W4), "band6": band_mask(W6),
        "cb4": corner_mask(W4), "cb6": corner_mask(W6), "ind6": ind6,
    }
    user = np.asarray(inputs["user_emb"], f32)
    item = np.asarray(inputs["item_emb"], f32)
    in_maps = []
    for b in range(8):
        m = dict(shared)
        m["xt"] = np.ascontiguousarray(item[b].T)
        m["ut"] = np.ascontiguousarray(user[b].T)
        in_maps.append(m)
    return in_maps


def kernel(**inputs):
    from concourse.bass_utils import run_bass_kernel_spmd
    if "nc" not in _CACHE:
        _CACHE["nc"] = _build_program()
    nc = _CACHE["nc"]
    in_maps = _host_prep(inputs)
    res = run_bass_kernel_spmd(nc, in_maps, list(range(8)))
    out = np.stack([res.results[b]["out"] for b in range(8)], axis=1)
    return out.astype(np.float32)


# revision 7
# speedup vs baseline: 2.2364x; 2.2364x over previous
"""Trainium2 Bass kernel for nn_CGRModel (6-branch MHA + PLE gates + 3 heads).

Self-contained: hardcodes all shapes. Data-parallel over batch B=8 across the
8 NeuronCores (1 sequence per core). All activations are kept transposed
[feature, L] on chip; weights are host-pre-transposed to [in, out] so weight
tiles serve directly as the matmul stationary operand. Attention scores are
computed transposed (S^T: k on partitions, q on free) so softmax numerators
feed attn@V with no on-chip transposes; softmax skips max-subtraction (score
scale ~0.2); denominators are M=1 ones-column matmuls accumulated in PSUM.
"""
import numpy as np

P = 128
D = 512
L = 1024
NKT = 4    # feature tiles of 128 (D/P)
NLT = 2    # 512-wide L tiles
NMT = 8    # 128-wide L tiles
NPAIR = 4  # head pairs (8 heads, dh=64)
W4, W6 = 2, 3
BW4, BW6 = 130, 132  # band block widths (rounded to even for fp32r)

_CACHE = {}


def _blocks_for(kind, qt, W=None):
    if kind == "causal":
        out = []
        for kb in range(4 * qt + 4):
            qoff = max(0, 128 * (kb - 4 * qt))
            out.append((kb, qoff, 512 - qoff, "tri" if kb >= 4 * qt else None))
        return out
    if kind == "full":
        return [(kb, 0, 512, None) for kb in range(8)]
    if kind == "band":
        out = []
        for i, kb in enumerate(range(4 * qt, 4 * qt + 4)):
            qoff = 128 * i
            bw = BW4 if W == W4 else BW6
            out.append((kb, qoff, min(bw, 512 - qoff), "band"))
        return out
    raise ValueError(kind)


def _build_program():
    import concourse.mybir as mybir
    from concourse import bacc
    from concourse.tile import TileContext

    F32 = mybir.dt.float32
    F32R = mybir.dt.float32r
    AF = mybir.ActivationFunctionType

    nc = bacc.Bacc()
    dp = nc.declare_dram_parameter
    xt_p = dp("xt", [D, L], F32R, isOutput=False)
    ut_p = dp("ut", [D, L], F32R, isOutput=False)
    wqkv_p = dp("wqkv", [6, D, 3 * D], F32R, isOutput=False)
    wo_p = dp("wo", [6, D, D], F32R, isOutput=False)
    bqkv_p = dp("bqkv", [P, 72], F32, isOutput=False)
    bo_p = dp("bo", [6, 1, D], F32R, isOutput=False)
    bv_p = dp("bv", [6, 1, D], F32, isOutput=False)
    wcross_p = dp("wcross", [D, 2 * D], F32R, isOutput=False)
    bcross_p = dp("bcross", [P, 8], F32, isOutput=False)
    wgate_p = dp("wgate", [2, D, 6], F32R, isOutput=False)
    bg_p = dp("bg", [6, 2], F32, isOutput=False)
    wh1_p = dp("wh1", [3, D, 256], F32R, isOutput=False)
    bh1_p = dp("bh1", [P, 6], F32, isOutput=False)
    wh2_p = dp("wh2", [P, 6], F32R, isOutput=False)
    bh2_p = dp("bh2", [1, 3], F32, isOutput=False)
    ones_p = dp("ones", [P, 512], F32R, isOutput=False)
    tri_p = dp("tri", [P, P], F32R, isOutput=False)
    band4_p = dp("band4", [P, BW4], F32R, isOutput=False)
    band6_p = dp("band6", [P, BW6], F32R, isOutput=False)
    cb4_p = dp("cb4", [P, W4], F32, isOutput=False)
    cb6_p = dp("cb6", [P, W6], F32, isOutput=False)
    ind6_p = dp("ind6", [6, 6 * P], F32R, isOutput=False)
    out_p = dp("out", [3, L], F32, isOutput=True)

    with TileContext(nc) as tc:
        with (
            tc.tile_pool(name="sb", bufs=1) as sb,
            tc.tile_pool(name="ps", bufs=1, space="PSUM") as psp,
        ):
            # ---------------- constants + persistent activations ----------
            ones = sb.tile([P, 512], F32R, tag="ones")
            tri = sb.tile([P, P], F32R, tag="tri")
            band4 = sb.tile([P, BW4], F32R, tag="band4")
            band6 = sb.tile([P, BW6], F32R, tag="band6")
            cb4 = sb.tile([P, W4], F32, tag="cb4")
            cb6 = sb.tile([P, W6], F32, tag="cb6")
            ind6 = sb.tile([6, 6 * P], F32R, tag="ind6")
            bqkv = sb.tile([P, 72], F32, tag="bqkv")
            bcross = sb.tile([P, 8], F32, tag="bcross")
            bg6 = sb.tile([6, 2], F32, tag="bg6")
            bh1s = sb.tile([P, 6], F32, tag="bh1s")
            wh2s = sb.tile([P, 6], F32R, tag="wh2s")
            bh2s = sb.tile([1, 3], F32, tag="bh2s")
            for t_, p_ in ((ones, ones_p), (tri, tri_p), (band4, band4_p),
                           (band6, band6_p), (cb4, cb4_p), (cb6, cb6_p),
                           (ind6, ind6_p), (bqkv, bqkv_p), (bcross, bcross_p),
                           (bg6, bg_p), (bh1s, bh1_p), (wh2s, wh2_p),
                           (bh2s, bh2_p)):
                nc.sync.dma_start(out=t_, in_=p_[:, :])

            xt = [sb.tile([P, L], F32R, tag="xt", bufs=4, name=f"xt{k}")
                  for k in range(NKT)]
            for k in range(NKT):
                nc.sync.dma_start(out=xt[k], in_=xt_p[128 * k:128 * k + 128, :])
            hexp = [sb.tile([P, L], F32R, tag="hexp", bufs=4, name=f"hexp{k}")
                    for k in range(NKT)]
            hclk = [sb.tile([P, L], F32R, tag="hclk", bufs=4, name=f"hclk{k}")
                    for k in range(NKT)]
            htasks = [hexp, hclk]

            # ---------------- gates ---------------------------------------
            g6 = []
            for t in range(2):
                egs = []
                wg = [sb.tile([P, 6], F32R, tag="wg", bufs=8, name=f"wg{t}{k}")
                      for k in range(NKT)]
                for k in range(NKT):
                    nc.sync.dma_start(out=wg[k],
                                      in_=wgate_p[t, 128 * k:128 * k + 128, :])
                for lt in range(NLT):
                    gps = psp.tile([P, 512], F32, tag="mm", bufs=2,
                                   name=f"gps{t}{lt}")
                    for k in range(NKT):
                        nc.tensor.matmul(gps[0:6, 0:512], wg[k][:, 0:6],
                                         xt[k][:, 512 * lt:512 * lt + 512],
                                         start=(k == 0), stop=(k == NKT - 1))
                    eg = sb.tile([6, 512], F32R, tag="eg", bufs=2,
                                 name=f"eg{t}{lt}")
                    egs.append(eg)
                    nc.scalar.activation(eg[0:6, :], gps[0:6, 0:512], AF.Exp,
                                         bias=bg6[0:6, t:t + 1])
                dg = psp.tile([1, L], F32, tag="den", bufs=1, name=f"dg{t}")
                for lt in range(NLT):
                    nc.tensor.matmul(dg[0:1, 512 * lt:512 * lt + 512],
                                     ones[0:6, 0:1], egs[lt][0:6, :],
                                     start=True, stop=True,
                                     tile_position=(0, 0))
                rg = sb.tile([1, L], F32, tag="rg", bufs=1, name=f"rg{t}")
                nc.vector.reciprocal_approx_fast(rg[0:1, :], dg[0:1, :])
                g6t = sb.tile([6, L], F32R, tag="g6", bufs=2, name=f"g6{t}")
                for lt in range(NLT):
                    rgb = sb.tile([6, 512], F32, tag="rgb", bufs=1,
                                  name=f"rgb{t}{lt}")
                    nc.gpsimd.partition_broadcast(
                        rgb[0:6, :], rg[0:1, 512 * lt:512 * lt + 512])
                    nc.vector.tensor_mul(g6t[0:6, 512 * lt:512 * lt + 512],
                                         egs[lt][0:6, :], rgb[0:6, :])
                g6.append(g6t)

            # ---------------- helpers -------------------------------------
            def emit_gating(expert, e):
                for t in range(2):
                    for lt in range(NLT):
                        ls = slice(512 * lt, 512 * lt + 512)
                        bcp = psp.tile([P, 512], F32, tag="mm", bufs=2,
                                       name=f"gb{e}{t}{lt}")
                        nc.tensor.matmul(bcp[:, 0:512],
                                         ind6[0:6, 128 * e:128 * e + 128],
                                         g6[t][0:6, ls], start=True, stop=True,
                                         tile_position=(0, 0))
                        for k in range(NKT):
                            if e == 0:
                                nc.vector.tensor_mul(htasks[t][k][:, ls],
                                                     expert[k][:, ls], bcp)
                            else:
                                tmp = sb.tile([P, 512], F32, tag="e", bufs=3,
                                              name=f"gt{e}{t}{lt}{k}")
                                nc.vector.tensor_mul(tmp, expert[k][:, ls], bcp)
                                nc.vector.tensor_add(htasks[t][k][:, ls],
                                                     htasks[t][k][:, ls], tmp)

            def emit_attn(a, kind, qsrc, kvsrc, resid, out_tiles, e, W=None):
                # weights
                w_sb = [sb.tile([P, 3 * D], F32R, tag="wqkv", bufs=4,
                                name=f"wqkv{a}_{k}") for k in range(NKT)]
                wo_sb = [sb.tile([P, D], F32R, tag="wo", bufs=4,
                                 name=f"wo{a}_{k}") for k in range(NKT)]
                for k in range(NKT):
                    nc.sync.dma_start(
                        out=w_sb[k], in_=wqkv_p[a, 128 * k:128 * k + 128, :])
                    nc.sync.dma_start(
                        out=wo_sb[k], in_=wo_p[a, 128 * k:128 * k + 128, :])
                bor = sb.tile([1, D], F32R, tag="bor", bufs=1, name=f"bor{a}")
                nc.sync.dma_start(out=bor, in_=bo_p[a])
                bvr = sb.tile([1, D], F32, tag="bvr", bufs=1, name=f"bvr{a}")
                nc.sync.dma_start(out=bvr, in_=bv_p[a])
                bvbc = sb.tile([P, 512], F32, tag="bc", bufs=2,
                               name=f"bvbc{a}")
                nc.gpsimd.partition_broadcast(bvbc[:, :], bvr[0:1, :])

                # V in natural layout [l, feat]
                vnat = []
                for mt in range(NMT):
                    vps = psp.tile([P, 512], F32, tag="mm", bufs=2,
                                   name=f"vps{a}{mt}")
                    for k in range(NKT):
                        nc.tensor.matmul(
                            vps, kvsrc[k][:, 128 * mt:128 * mt + 128],
                            w_sb[k][:, 2 * D:3 * D],
                            start=(k == 0), stop=(k == NKT - 1))
                    vt = sb.tile([P, 512], F32R, tag="v", bufs=8,
                                 name=f"v{a}{mt}")
                    nc.vector.tensor_add(vt, vps, bvbc)
                    vnat.append(vt)

                mask_t = {"tri": tri, "band": band4 if W == W4 else band6}
                cmask = cb4 if W == W4 else cb6
                cat = []
                for t in range(NPAIR):
                    # Q^T / K^T for this head pair (feature rows 128t..)
                    qta = sb.tile([P, L], F32R, tag="qk", bufs=4,
                                  name=f"q{a}{t}")
                    kta = sb.tile([P, L], F32R, tag="qk", bufs=4,
                                  name=f"k{a}{t}")
                    for dst, coff, bcol, src in (
                            (qta, 128 * t, 12 * a + t, qsrc),
                            (kta, D + 128 * t, 12 * a + 4 + t, kvsrc)):
                        for lt in range(NLT):
                            ls = slice(512 * lt, 512 * lt + 512)
                            ips = psp.tile([P, 512], F32, tag="mm", bufs=2,
                                           name=f"ip{a}{t}{lt}")
                            for k in range(NKT):
                                nc.tensor.matmul(
                                    ips, w_sb[k][:, coff:coff + 128],
                                    src[k][:, ls],
                                    start=(k == 0), stop=(k == NKT - 1))
                            nc.vector.tensor_scalar_add(
                                dst[:, ls], ips, bqkv[:, bcol:bcol + 1])
                    ct = sb.tile([P, L], F32R, tag="cat", bufs=4,
                                 name=f"cat{a}{t}")
                    cat.append(ct)
                    for qt in range(2):
                        blocks = _blocks_for(kind, qt, W)
                        corner = (kind == "band" and qt == 1)
                        ngrp = len(blocks) + (1 if corner else 0)
                        U = psp.tile([P, 512], F32, tag="u", bufs=1,
                                     name=f"U{a}{t}{qt}")
                        UB = psp.tile([P, 512], F32, tag="ub", bufs=1,
                                      name=f"UB{a}{t}{qt}")
                        den = psp.tile([1, L], F32, tag="den", bufs=1,
                                       name=f"den{a}{t}{qt}")
                        for bi, (kb, qoff, w, mk) in enumerate(blocks):
                            st = bi == 0
                            sp = bi == ngrp - 1
                            qs = slice(512 * qt + qoff, 512 * qt + qoff + w)
                            sA = psp.tile([P, 512], F32, tag="s", bufs=2,
                                          name=f"sA{a}{t}{qt}{kb}")
                            sB = psp.tile([P, 512], F32, tag="s", bufs=2,
                                          name=f"sB{a}{t}{qt}{kb}")
                            ks = slice(128 * kb, 128 * kb + 128)
                            nc.tensor.matmul(sA[:, 0:w], kta[0:64, ks],
                                             qta[0:64, qs],
                                             start=True, stop=True)
                            nc.tensor.matmul(sB[:, 0:w], kta[64:128, ks],
                                             qta[64:128, qs],
                                             start=True, stop=True,
                                             tile_position=(64, 0))
                            eA = sb.tile([P, 512], F32R, tag="e", bufs=3,
                                         name=f"eA{a}{t}{qt}{kb}")
                            eB = sb.tile([P, 512], F32R, tag="e", bufs=3,
                                         name=f"eB{a}{t}{qt}{kb}")
                            nc.scalar.activation(eA[:, 0:w], sA[:, 0:w], AF.Exp)
                            nc.scalar.activation(eB[:, 0:w], sB[:, 0:w], AF.Exp)
                            if mk == "tri":
                                nc.vector.tensor_mul(eA[:, 0:128],
                                                     eA[:, 0:128], tri)
                                nc.vector.tensor_mul(eB[:, 0:128],
                                                     eB[:, 0:128], tri)
                            elif mk == "band":
                                m = mask_t["band"]
                                nc.vector.tensor_mul(eA[:, 0:w], eA[:, 0:w],
                                                     m[:, 0:w])
                                nc.vector.tensor_mul(eB[:, 0:w], eB[:, 0:w],
                                                     m[:, 0:w])
                            qo = slice(qoff, qoff + w)
                            qo2 = slice(512 + qoff, 512 + qoff + w)
                            nc.tensor.matmul(U[0:64, qo],
                                             vnat[kb][:, 128 * t:128 * t + 64],
                                             eA[:, 0:w], start=st, stop=sp,
                                             tile_position=(0, 0))
                            nc.tensor.matmul(UB[0:64, qo],
                                             vnat[kb][:, 128 * t + 64:
                                                      128 * t + 128],
                                             eB[:, 0:w], start=st, stop=sp,
                                             tile_position=(0, 0))
                            nc.tensor.matmul(den[0:1, qo], ones[:, 0:1],
                                             eA[:, 0:w], start=st, stop=sp,
                                             tile_position=(0, 0))
                            nc.tensor.matmul(den[0:1, qo2], ones[:, 0:1],
                                             eB[:, 0:w], start=st, stop=sp,
                                             tile_position=(0, 0))
                        if corner:
                            Wc = W
                            sCA = psp.tile([P, 512], F32, tag="s", bufs=2,
                                           name=f"sCA{a}{t}")
                            sCB = psp.tile([P, 512], F32, tag="s", bufs=2,
                                           name=f"sCB{a}{t}")
                            ktaf = kta.bitcast(F32)
                            qtaf = qta.bitcast(F32)
                            vf = vnat[3].bitcast(F32)
                            onf = ones.bitcast(F32)
                            nc.tensor.matmul(sCA[96:128, 0:Wc],
                                             ktaf[0:64, 480:512],
                                             qtaf[0:64, 512:512 + Wc],
                                             start=True, stop=True,
                                             tile_position=(0, 96))
                            nc.tensor.matmul(sCB[96:128, 0:Wc],
                                             ktaf[64:128, 480:512],
                                             qtaf[64:128, 512:512 + Wc],
                                             start=True, stop=True,
                                             tile_position=(64, 96))
                            eCA = sb.tile([P, 8], F32, tag="ec", bufs=2,
                                          name=f"eCA{a}{t}")
                            eCB = sb.tile([P, 8], F32, tag="ec", bufs=2,
                                          name=f"eCB{a}{t}")
                            nc.scalar.activation(eCA[96:128, 0:Wc],
                                                 sCA[96:128, 0:Wc], AF.Exp)
                            nc.scalar.activation(eCB[96:128, 0:Wc],
                                                 sCB[96:128, 0:Wc], AF.Exp)
                            nc.vector.tensor_mul(eCA[96:128, 0:Wc],
                                                 eCA[96:128, 0:Wc],
                                                 cmask[96:128, 0:Wc])
                            nc.vector.tensor_mul(eCB[96:128, 0:Wc],
                                                 eCB[96:128, 0:Wc],
                                                 cmask[96:128, 0:Wc])
                            nc.tensor.matmul(
                                U[0:64, 0:Wc],
                                vf[96:128, 128 * t:128 * t + 64],
                                eCA[96:128, 0:Wc], start=False, stop=True,
                                tile_position=(96, 0))
                            nc.tensor.matmul(
                                UB[0:64, 0:Wc],
                                vf[96:128, 128 * t + 64:128 * t + 128],
                                eCB[96:128, 0:Wc], start=False, stop=True,
                                tile_position=(96, 0))
                            nc.tensor.matmul(den[0:1, 0:Wc],
                                             onf[96:128, 0:1],
                                             eCA[96:128, 0:Wc],
                                             start=False, stop=True,
                                             tile_position=(96, 0))
                            nc.tensor.matmul(den[0:1, 512:512 + Wc],
                                             onf[96:128, 0:1],
                                             eCB[96:128, 0:Wc],
                                             start=False, stop=True,
                                             tile_position=(96, 0))
                        rAB = sb.tile([1, L], F32, tag="ra", bufs=1,
                                      name=f"rAB{a}{t}{qt}")
                        nc.vector.reciprocal_approx_fast(rAB[0:1, :],
                                                         den[0:1, :])
                        bcA = sb.tile([P, 512], F32, tag="bc", bufs=2,
                                      name=f"bcA{a}{t}{qt}")
                        bcB = sb.tile([P, 512], F32, tag="bc", bufs=2,
                                      name=f"bcB{a}{t}{qt}")
                        nc.gpsimd.partition_broadcast(bcA[:, :], rAB[0:1, 0:512])
                        nc.gpsimd.partition_broadcast(bcB[:, :],
                                                      rAB[0:1, 512:1024])
                        qts = slice(512 * qt, 512 * qt + 512)
                        nc.vector.tensor_mul(ct[0:64, qts], U[0:64, 0:512],
                                             bcA[0:64, :])
                        nc.vector.tensor_mul(ct[64:128, qts], UB[0:64, 0:512],
                                             bcB[0:64, :])
                # out_proj (+bias via K=1 ones matmul) + residual
                for ot in range(NKT):
                    for lt in range(NLT):
                        ls = slice(512 * lt, 512 * lt + 512)
                        ops = psp.tile([P, 512], F32, tag="mm", bufs=2,
                                       name=f"op{a}{ot}{lt}")
                        for k in range(NKT):
                            nc.tensor.matmul(
                                ops, wo_sb[k][:, 128 * ot:128 * ot + 128],
                                cat[k][:, ls],
                                start=(k == 0), stop=False)
                        nc.tensor.matmul(ops, bor[0:1, 128 * ot:128 * ot + 128],
                                         ones[0:1, 0:512],
                                         start=False, stop=True,
                                         tile_position=(0, 0))
                        if resid is not None:
                            nc.vector.tensor_add(out_tiles[ot][:, ls], ops,
                                                 resid[ot][:, ls])
                        else:
                            nc.vector.tensor_copy(out_tiles[ot][:, ls], ops)
                emit_gating(out_tiles, e)

            # ---------------- the six attentions --------------------------
            sharedT = [sb.tile([P, L], F32R, tag="big", bufs=4,
                               name=f"sh{k}") for k in range(NKT)]
            emit_attn(0, "causal", xt, xt, xt, sharedT, 0)
            e4 = [sb.tile([P, L], F32, tag="exp", bufs=4, name=f"e4_{k}")
                  for k in range(NKT)]
            emit_attn(4, "band", xt, xt, xt, e4, 4, W=W4)
            e1 = [sb.tile([P, L], F32, tag="exp", bufs=4, name=f"e1_{k}")
                  for k in range(NKT)]
            emit_attn(1, "causal", sharedT, sharedT, sharedT, e1, 1)
            e5 = [sb.tile([P, L], F32, tag="exp", bufs=4, name=f"e5_{k}")
                  for k in range(NKT)]
            emit_attn(5, "band", xt, xt, xt, e5, 5, W=W6)
            e2 = [sb.tile([P, L], F32, tag="exp", bufs=4, name=f"e2_{k}")
                  for k in range(NKT)]
            emit_attn(2, "causal", sharedT, sharedT, sharedT, e2, 2)

            # cross projections u = user @ cuW.T + b ; pv = item @ ciW.T + b
            ut = [sb.tile([P, L], F32R, tag="exp", bufs=4, name=f"ut{k}")
                  for k in range(NKT)]
            for k in range(NKT):
                nc.sync.dma_start(out=ut[k], in_=ut_p[128 * k:128 * k + 128, :])
            wc_sb = [sb.tile([P, 2 * D], F32R, tag="wqkv", bufs=4,
                             name=f"wc{k}") for k in range(NKT)]
            for k in range(NKT):
                nc.sync.dma_start(out=wc_sb[k],
                                  in_=wcross_p[128 * k:128 * k + 128, :])
            def emit_cross_proj(dst, coff, bcol, src):
                for ot in range(NKT):
                    for lt in range(NLT):
                        ls = slice(512 * lt, 512 * lt + 512)
                        cps = psp.tile([P, 512], F32, tag="mm", bufs=2,
                                       name=f"cp{coff}{ot}{lt}")
                        for k in range(NKT):
                            nc.tensor.matmul(
                                cps,
                                wc_sb[k][:, coff + 128 * ot:coff + 128 * ot + 128],
                                src[k][:, ls],
                                start=(k == 0), stop=(k == NKT - 1))
                        nc.vector.tensor_scalar_add(
                            dst[ot][:, ls], cps,
                            bcross[:, bcol + ot:bcol + ot + 1])

            uT = [sb.tile([P, L], F32R, tag="big", bufs=4, name=f"uT{k}")
                  for k in range(NKT)]
            emit_cross_proj(uT, 0, 0, ut)
            pvT = [sb.tile([P, L], F32R, tag="exp", bufs=4, name=f"pvT{k}")
                   for k in range(NKT)]
            emit_cross_proj(pvT, D, 4, xt)
            e3 = [sb.tile([P, L], F32, tag="exp", bufs=4, name=f"e3_{k}")
                  for k in range(NKT)]
            emit_attn(3, "full", uT, pvT, None, e3, 3)

            # ---------------- heads ---------------------------------------
            for i, hsrc in ((0, hexp), (1, hclk), (2, hclk)):
                wh1 = [sb.tile([P, 256], F32R, tag="wo", bufs=4,
                               name=f"wh1_{i}{k}") for k in range(NKT)]
                for k in range(NKT):
                    nc.sync.dma_start(out=wh1[k],
                                      in_=wh1_p[i, 128 * k:128 * k + 128, :])
                hid = [sb.tile([P, L], F32R, tag="cat", bufs=4,
                               name=f"hid{i}{m}") for m in range(2)]
                for mt in range(2):
                    for lt in range(NLT):
                        ls = slice(512 * lt, 512 * lt + 512)
                        hps = psp.tile([P, 512], F32, tag="mm", bufs=2,
                                       name=f"hp{i}{mt}{lt}")
                        for k in range(NKT):
                            nc.tensor.matmul(
                                hps, wh1[k][:, 128 * mt:128 * mt + 128],
                                hsrc[k][:, ls],
                                start=(k == 0), stop=(k == NKT - 1))
                        nc.scalar.activation(hid[mt][:, ls], hps, AF.Relu,
                                             bias=bh1s[:, 2 * i + mt:
                                                       2 * i + mt + 1])
                pps = psp.tile([1, L], F32, tag="den", bufs=1, name=f"pp{i}")
                for lt in range(NLT):
                    ls = slice(512 * lt, 512 * lt + 512)
                    for mt in range(2):
                        nc.tensor.matmul(pps[0:1, ls],
                                         wh2s[:, mt * 3 + i:mt * 3 + i + 1],
                                         hid[mt][:, ls],
                                         start=(mt == 0), stop=(mt == 1),
                                         tile_position=(0, 0))
                psb = sb.tile([1, L], F32, tag="p", bufs=1, name=f"psb{i}")
                nc.scalar.activation(psb[0:1, :], pps[0:1, :], AF.Sigmoid,
                                     bias=bh2s[0:1, i:i + 1])
                nc.sync.dma_start(out=out_p[i:i + 1, :], in_=psb)

    nc.finalize()
    return nc


def _host_prep(inputs):
    """Build the shared (weight/const) input map pieces + per-core slices."""
    f32 = np.float32
    w_in = np.asarray(inputs["attn_w_in"], f32)      # [6, 1536, 512]
    b_in = np.asarray(inputs["attn_b_in"], f32)      # [6, 1536]
    w_out = np.asarray(inputs["attn_w_out"], f32)    # [6, 512, 512]
    b_out = np.asarray(inputs["attn_b_out"], f32)    # [6, 512]

    wqkv = np.ascontiguousarray(np.transpose(w_in, (0, 2, 1)))  # [6,512,1536]
    wqkv[:, :, 0:512] /= 8.0                         # fold 1/sqrt(dh) into Wq
    bqkv = b_in.reshape(6, 12, 128)                  # [6, 12f, 128]
    bqkv = bqkv.copy()
    bqkv[:, 0:4, :] /= 8.0
    # device layout [128, 72]: col a*12 + j  (j = feature-tile: 0-3 q, 4-7 k, 8-11 v)
    bqkv_dev = np.ascontiguousarray(bqkv.transpose(2, 0, 1).reshape(128, 72))

    wo = np.ascontiguousarray(np.transpose(w_out, (0, 2, 1)))   # [6,512,512]
    bo = np.ascontiguousarray(b_out[:, None, :])                # [6,1,512]
    bv = np.ascontiguousarray(b_in[:, None, 1024:1536])         # [6,1,512]

    cu = np.asarray(inputs["cross_user_w"], f32)
    ci = np.asarray(inputs["cross_item_w"], f32)
    wcross = np.ascontiguousarray(np.concatenate([cu.T, ci.T], axis=1))
    bcross = np.ascontiguousarray(
        np.concatenate([np.asarray(inputs["cross_user_b"], f32)
                        .reshape(4, 128).T,
                        np.asarray(inputs["cross_item_b"], f32)
                        .reshape(4, 128).T], axis=1))

    wgate = np.ascontiguousarray(np.asarray(inputs["gate_w"], f32))
    bg = np.ascontiguousarray(np.asarray(inputs["gate_b"], f32).T)  # [6,2]
    wh1 = np.ascontiguousarray(np.asarray(inputs["head_w1"], f32))
    bh1 = np.ascontiguousarray(
        np.asarray(inputs["head_b1"], f32).reshape(3, 2, 128)
        .transpose(2, 0, 1).reshape(128, 6))
    # wh2 device [128, 6]: col kt*3 + i
    wh2 = np.ascontiguousarray(
        np.asarray(inputs["head_w2"], f32).reshape(3, 2, 128)
        .transpose(2, 1, 0).reshape(128, 6))
    bh2 = np.ascontiguousarray(np.asarray(inputs["head_b2"], f32)[None, :])

    k_i = np.arange(128)[:, None]
    tri = (np.arange(128)[None, :] >= k_i).astype(f32)
    def band_mask(W):
        bw = BW4 if W == W4 else BW6
        c = np.arange(bw)[None, :]
        return ((c - k_i >= 0) & (c - k_i <= W)).astype(f32)
    def corner_mask(W):
        m = np.zeros((128, W), f32)
        r = np.arange(96, 128)[:, None] - 96
        c = np.arange(W)[None, :]
        m[96:128, :] = (32 + c - r <= W).astype(f32)
        return m
    ind6 = np.zeros((6, 768), f32)
    for e in range(6):
        ind6[e, 128 * e:128 * e + 128] = 1.0

    shared = {
        "wqkv": wqkv, "bqkv": bqkv_dev, "wo": wo, "bo": bo, "bv": bv,
        "wcross": wcross, "bcross": bcross, "wgate": wgate, "bg": bg,
        "wh1": wh1, "bh1": bh1, "wh2": wh2, "bh2": bh2,
        "ones": np.ones((128, 512), f32), "tri": tri,
        "band4": band_mask(W4), "band6": band_mask(W6),
        "cb4": corner_mask(W4), "cb6": corner_mask(W6), "ind6": ind6,
    }
    user = np.asarray(inputs["user_emb"], f32)
    item = np.asarray(inputs["item_emb"], f32)
    in_maps = []
    for b in range(8):
        m = dict(shared)
        m["xt"] = np.ascontiguousarray(item[b].T)
        m["ut"] = np.ascontiguousarray(user[b].T)
        in_maps.append(m)
    return in_maps


def kernel(**inputs):
    from concourse.bass_utils import run_bass_kernel_spmd
    if "nc" not in _CACHE:
        _CACHE["nc"] = _build_program()
    nc = _CACHE["nc"]
    in_maps = _host_prep(inputs)
    res = run_bass_kernel_spmd(nc, in_maps, list(range(8)))
    out = np.stack([res.results[b]["out"] for b in range(8)], axis=1)
    return out.astype(np.float32)
